# revision 1
# baseline (speedup 1.0000x reference)
"""Multi-head self-attention (B=4, S=2048, hidden=1024, 16 heads, d_k=64,
causal) on 8 Trainium2 NeuronCores.

Sharding: core c handles batch b = c//2 and head-group hg = c%2 (8 heads =
512 hidden dims). Each core computes Q/K/V for its heads, causal attention,
and a partial output projection against its wo column-slice; the host sums
the two partials per batch and adds bo.

Device layouts (SBUF is [128 partitions, free]):
  x^T   [in=8*128, tok]      host-transposed activations
  Q^T/K^T [dout=4*128, tok]  head h occupies rows h*64..h*64+64
  V     [tok, head, d_k+1]   65th column is ones so the PV matmul also
                             produces the softmax denominator row
  scores are computed transposed: S^T[k, q] = K @ Q^T, so softmax's sum
  over keys becomes a matmul contraction instead of a partition reduction.
"""

import os
import sys

for _p in (
    "/root/.axon_site",
    "/root/.axon_site/_ro/trn_rl_repo",
    "/root/.axon_site/_ro/pypackages",
    "/opt/trn_rl_repo",
):
    if os.path.isdir(_p) and _p not in sys.path:
        sys.path.append(_p)

import numpy as np

import concourse.mybir as mybir
import concourse.tile as tile
from concourse import bacc
from concourse.bass import ts
from concourse.bass_utils import run_bass_kernel_spmd

F32 = mybir.dt.float32
F32R = mybir.dt.float32r  # fp32 storage, single-pass (bf16-rounded) matmul
AF = mybir.ActivationFunctionType
ALU = mybir.AluOpType

USE_F32R = True           # fast matmuls (~1.4e-4 rel err vs ~1e-7 for fp32)
MDT = F32R if USE_F32R else F32

B, S, HID = 4, 2048, 1024
HEADS, DK = 16, 64
NCORES = 8
HPC = HEADS // 2          # 8 heads per core
HSL = HPC * DK            # 512-dim hidden slice per core
TC = 512                  # token/query chunk
NTC = S // TC             # 4
NTB = S // 128            # 16 token blocks
NEG = -1.0e30


def build_nc(debug_dumps=False, att_f32r=True, fast_norm=True, grp=3):
    def att_mm(out, lhsT, rhs, **kw):
        if att_f32r:
            nc.tensor.matmul(out, lhsT, rhs, **kw)
        else:
            nc.tensor.matmul(
                out, lhsT.bitcast(F32), rhs.bitcast(F32), **kw
            )

    nc = bacc.Bacc("TRN2", target_bir_lowering=False, debug=False)

    xT = nc.dram_tensor("xT", [HID, S], MDT, kind="ExternalInput").ap()
    wqT = nc.dram_tensor("wqT", [HID, HSL], MDT, kind="ExternalInput").ap()
    wkT = nc.dram_tensor("wkT", [HID, HSL], MDT, kind="ExternalInput").ap()
    wvT = nc.dram_tensor("wvT", [HID, HSL], MDT, kind="ExternalInput").ap()
    woT = nc.dram_tensor("woT", [HSL, HID], MDT, kind="ExternalInput").ap()
    bq = nc.dram_tensor("bq", [HSL], F32, kind="ExternalInput").ap()
    bk = nc.dram_tensor("bk", [HSL], F32, kind="ExternalInput").ap()
    bv_rep = nc.dram_tensor("bv_rep", [128, HSL], F32, kind="ExternalInput").ap()
    dmask = nc.dram_tensor("dmask", [128, 128], F32, kind="ExternalInput").ap()
    out = nc.dram_tensor("out_p", [S, HID], F32, kind="ExternalOutput").ap()
    if debug_dumps:
        qT_d = nc.dram_tensor("qT_d", [HSL, S], F32, kind="ExternalOutput").ap()
        kT_d = nc.dram_tensor("kT_d", [HSL, S], F32, kind="ExternalOutput").ap()
        v_d = nc.dram_tensor(
            "v_d", [128, NTB * HPC * (DK + 1)], F32, kind="ExternalOutput"
        ).ap()
        aT_d = nc.dram_tensor("aT_d", [HSL, S], F32, kind="ExternalOutput").ap()

    xT_r = xT.rearrange("(ic p) t -> p ic t", p=128)      # [128, 8, S]
    wqT_r = wqT.rearrange("(ic p) o -> p ic o", p=128)    # [128, 8, HSL]
    wkT_r = wkT.rearrange("(ic p) o -> p ic o", p=128)
    wvT_r = wvT.rearrange("(ic p) o -> p ic o", p=128)
    woT_r = woT.rearrange("(hb p) o -> p hb o", p=128)    # [128, 4, HID]
    bq_r = bq.rearrange("(d p) -> p d", p=128)            # [128, 4]
    bk_r = bk.rearrange("(d p) -> p d", p=128)
    out_r = out.rearrange("(tb p) o -> p tb o", p=128)    # [128, 16, HID]

    with tile.TileContext(nc) as tc:
        with tc.tile_pool(name="qkv", bufs=1) as qkv_pool:
            qT_sb = qkv_pool.tile([128, 4, S], MDT, tag="qT")
            kT_sb = qkv_pool.tile([128, 4, S], MDT, tag="kT")
            v_sb = qkv_pool.tile([128, NTB, HPC, DK + 1], MDT, tag="v")
            nc.vector.memset(v_sb[:, :, :, DK].bitcast(F32), 1.0)

            # ---------------- QKV projections ----------------
            with (
                tc.tile_pool(name="pw", bufs=1) as pw,
                tc.tile_pool(name="xt", bufs=2) as xt_pool,
                tc.tile_pool(name="psq", bufs=4, space="PSUM") as psq,
            ):
                wq_sb = pw.tile([128, 8, HSL], MDT, tag="wq")
                nc.sync.dma_start(wq_sb[:], wqT_r)
                wk_sb = pw.tile([128, 8, HSL], MDT, tag="wk")
                nc.sync.dma_start(wk_sb[:], wkT_r)
                wv_sb = pw.tile([128, 8, HSL], MDT, tag="wv")
                nc.sync.dma_start(wv_sb[:], wvT_r)
                bq_sb = pw.tile([128, 4], F32, tag="bq")
                nc.sync.dma_start(bq_sb[:], bq_r)
                bk_sb = pw.tile([128, 4], F32, tag="bk")
                nc.sync.dma_start(bk_sb[:], bk_r)
                bv_sb = pw.tile([128, HSL], F32, tag="bv")
                nc.sync.dma_start(bv_sb[:], bv_rep)

                for tci in range(NTC):
                    xt = xt_pool.tile([128, 8, TC], MDT, tag="xt")
                    nc.sync.dma_start(xt[:], xT_r[:, :, ts(tci, TC)])
                    for w_sb, b_sb, dst in (
                        (wq_sb, bq_sb, qT_sb),
                        (wk_sb, bk_sb, kT_sb),
                    ):
                        for dblk in range(4):
                            ps = psq.tile([128, TC], F32, tag="ps")
                            for ic in range(8):
                                nc.tensor.matmul(
                                    ps[:],
                                    w_sb[:, ic, ts(dblk, 128)],
                                    xt[:, ic, :],
                                    start=(ic == 0),
                                    stop=(ic == 7),
                                )
                            nc.vector.tensor_tensor(
                                dst[:, dblk, ts(tci, TC)],
                                ps[:],
                                b_sb[:, dblk : dblk + 1].to_broadcast((128, TC)),
                                ALU.add,
                            )
                    for tbl in range(4):
                        ps = psq.tile([128, TC], F32, tag="ps")
                        for ic in range(8):
                            nc.tensor.matmul(
                                ps[:],
                                xt[:, ic, ts(tbl, 128)],
                                wv_sb[:, ic, :],
                                start=(ic == 0),
                                stop=(ic == 7),
                            )
                        tb = tci * 4 + tbl
                        nc.vector.tensor_tensor(
                            v_sb[:, tb, :, 0:DK],
                            ps.rearrange("p (h d) -> p h d", d=DK),
                            bv_sb.rearrange("p (h d) -> p h d", d=DK),
                            ALU.add,
                        )

            # ---------------- attention + output projection ----------------
            with tc.tile_pool(name="att", bufs=1) as att_pool:
                aT_sb = att_pool.tile([128, 4, S], MDT, tag="aT")
                wo_sb = att_pool.tile([128, 4, HID], MDT, tag="wo")
                nc.sync.dma_start(wo_sb[:], woT_r)
                dm_sb = att_pool.tile([128, 128], F32, tag="dm")
                nc.sync.dma_start(dm_sb[:], dmask)

                GRP = grp  # k-blocks per PSUM score group (banks)
                with (
                    tc.tile_pool(name="pt", bufs=4) as pt_pool,
                    tc.tile_pool(name="sm", bufs=3) as sm_pool,
                    tc.tile_pool(name="pss", bufs=2, space="PSUM") as pss,
                    tc.tile_pool(name="pso", bufs=2, space="PSUM") as pso,
                ):
                    for h in range(HPC):
                        dblk, off = h // 2, (h % 2) * DK
                        qT_h = qT_sb[off : off + DK, dblk]
                        kT_h = kT_sb[off : off + DK, dblk]
                        for qc in range(NTC):
                            nkb = 4 * qc + 4
                            ops = pso.tile([DK + 1, TC], F32, tag="ops")
                            pend = None

                            def emit_pv(pt_tile, kbs):
                                for j, kb in enumerate(kbs):
                                    cs = max(0, kb * 128 - qc * TC)
                                    att_mm(
                                        ops[:, cs:TC],
                                        v_sb[:, kb, h, :],
                                        pt_tile[:, j, cs:TC],
                                        start=(kb == 0),
                                        stop=(kb == nkb - 1),
                                    )

                            for g0 in range(0, nkb, GRP):
                                kbs = tuple(range(g0, min(g0 + GRP, nkb)))
                                sp = pss.tile([128, GRP, TC], F32, tag="sp")
                                pt = pt_pool.tile([128, GRP, TC], MDT, tag="pt")
                                for j, kb in enumerate(kbs):
                                    cs = max(0, kb * 128 - qc * TC)
                                    att_mm(
                                        sp[:, j, cs:TC],
                                        kT_h[:, ts(kb, 128)],
                                        qT_h[:, qc * TC + cs : (qc + 1) * TC],
                                        start=True,
                                        stop=True,
                                    )
                                    if kb >= 4 * qc:  # diagonal: causal mask
                                        nc.vector.tensor_tensor(
                                            sp[:, j, cs : cs + 128],
                                            sp[:, j, cs : cs + 128],
                                            dm_sb[:],
                                            ALU.add,
                                        )

                                if kbs[-1] > 4 * qc:  # group hits the diagonal
                                    for j, kb in enumerate(kbs):
                                        cs = max(0, kb * 128 - qc * TC)
                                        nc.scalar.activation(
                                            pt[:, j, cs:TC],
                                            sp[:, j, cs:TC],
                                            AF.Exp,
                                            scale=0.125,
                                        )
                                else:
                                    nc.scalar.activation(
                                        pt[:, 0 : len(kbs), :],
                                        sp[:, 0 : len(kbs), :],
                                        AF.Exp,
                                        scale=0.125,
                                    )
                                if pend is not None:
                                    emit_pv(*pend)
                                pend = (pt, kbs)
                            emit_pv(*pend)

                            rc = sm_pool.tile([1, TC], F32, tag="rc")
                            if fast_norm:
                                # custom-DVE ops mishandle partition-offset
                                # inputs: stage the sums row at partition 0
                                lsb = sm_pool.tile([1, TC], F32, tag="lsb")
                                nc.vector.tensor_copy(lsb[:], ops[DK : DK + 1, :])
                                nc.vector.reciprocal_approx_fast(rc[:], lsb[:])
                            else:
                                nc.vector.reciprocal(rc[:], ops[DK : DK + 1, :])
                            bcs = sm_pool.tile([DK, TC], F32, tag="bcs")
                            nc.gpsimd.partition_broadcast(bcs[:], rc[:])
                            if off == 0:
                                nc.vector.tensor_tensor(
                                    aT_sb[0:DK, dblk, ts(qc, TC)],
                                    ops[0:DK, :],
                                    bcs[:],
                                    ALU.mult,
                                )
                            else:
                                tmp = sm_pool.tile([DK, TC], MDT, tag="tmp")
                                nc.vector.tensor_tensor(
                                    tmp[:], ops[0:DK, :], bcs[:], ALU.mult
                                )
                                # engines are lane-locked; DMA shifts partitions
                                nc.sync.dma_start(aT_sb[DK:128, dblk, ts(qc, TC)], tmp[:])

                # ---------------- output projection ----------------
                with (
                    tc.tile_pool(name="ot", bufs=3) as ot_pool,
                    tc.tile_pool(name="psp", bufs=4, space="PSUM") as psp,
                ):
                    for tb in range(NTB):
                        ot = ot_pool.tile([128, HID], F32, tag="ot")
                        for half in range(2):
                            ps = psp.tile([128, 512], F32, tag="ps2")
                            for hb in range(4):
                                nc.tensor.matmul(
                                    ps[:],
                                    aT_sb[:, hb, ts(tb, 128)],
                                    wo_sb[:, hb, ts(half, 512)],
                                    start=(hb == 0),
                                    stop=(hb == 3),
                                )
                            nc.vector.tensor_copy(ot[:, ts(half, 512)], ps[:])
                        nc.sync.dma_start(out_r[:, tb, :], ot[:])

            if debug_dumps:
                nc.sync.dma_start(
                    qT_d.rearrange("(d p) t -> p d t", p=128),
                    qT_sb[:].bitcast(F32),
                )
                nc.sync.dma_start(
                    kT_d.rearrange("(d p) t -> p d t", p=128),
                    kT_sb[:].bitcast(F32),
                )
                nc.sync.dma_start(
                    v_d[:], v_sb.rearrange("p a b c -> p (a b c)").bitcast(F32)
                )
                nc.sync.dma_start(
                    aT_d.rearrange("(d p) t -> p d t", p=128),
                    aT_sb[:].bitcast(F32),
                )
    nc.compile()
    return nc


_NC = None


def _get_nc():
    global _NC
    if _NC is None:
        _NC = build_nc()
    return _NC


def _numpy_reference(x, attn_mask, wq, bq, wk, bk, wv, bv, wo, bo):
    """Fallback for a non-causal mask (never hit with the standard inputs)."""
    Bsz, Seq, D = x.shape
    scale = 1.0 / np.sqrt(DK)

    def proj(w, b):
        y = x @ w.T + b
        return y.reshape(Bsz, Seq, HEADS, DK).transpose(0, 2, 1, 3)

    q, k, v = proj(wq, bq), proj(wk, bk), proj(wv, bv)
    scores = np.einsum("bhqd,bhkd->bhqk", q, k) * scale
    scores = np.where(attn_mask == 0, np.float32(-1e9), scores)
    scores = scores - scores.max(axis=-1, keepdims=True)
    p = np.exp(scores)
    p /= p.sum(axis=-1, keepdims=True)
    o = np.einsum("bhqk,bhkd->bhqd", p, v)
    o = o.transpose(0, 2, 1, 3).reshape(Bsz, Seq, D)
    return o @ wo.T + bo


def kernel(x, attn_mask, wq, bq, wk, bk, wv, bv, wo, bo, **_unused):
    x = np.asarray(x, np.float32)
    attn_mask = np.asarray(attn_mask)
    wq, bq = np.asarray(wq, np.float32), np.asarray(bq, np.float32)
    wk, bk = np.asarray(wk, np.float32), np.asarray(bk, np.float32)
    wv, bv = np.asarray(wv, np.float32), np.asarray(bv, np.float32)
    wo, bo = np.asarray(wo, np.float32), np.asarray(bo, np.float32)

    causal = np.array_equal(
        np.asarray(attn_mask).reshape(S, S) != 0, np.tril(np.ones((S, S), bool))
    )
    if not causal:
        return _numpy_reference(x, attn_mask, wq, bq, wk, bk, wv, bv, wo, bo)

    def r32r(a):
        # round fp32 to the PE's fp32r grid (nearest, 12 low mantissa bits
        # dropped) — required for tensors consumed by fp32r matmuls
        b = np.ascontiguousarray(a, np.float32).view(np.uint32)
        return ((b + 0x800) & 0xFFFFF000).view(np.float32)

    if not USE_F32R:
        r32r = lambda a: np.ascontiguousarray(a, np.float32)  # noqa: E731

    tri = np.where(
        np.arange(128)[:, None] <= np.arange(128)[None, :], 0.0, NEG
    ).astype(np.float32)

    in_maps = []
    for c in range(NCORES):
        b, hg = c // 2, c % 2
        sl = slice(hg * HSL, (hg + 1) * HSL)
        in_maps.append(
            {
                "xT": r32r(x[b].T),
                "wqT": r32r(wq[sl, :].T),
                "wkT": r32r(wk[sl, :].T),
                "wvT": r32r(wv[sl, :].T),
                "woT": r32r(wo[:, sl].T),
                "bq": np.ascontiguousarray(bq[sl]),
                "bk": np.ascontiguousarray(bk[sl]),
                "bv_rep": np.tile(bv[sl][None, :], (128, 1)),
                "dmask": tri,
            }
        )

    res = run_bass_kernel_spmd(
        _get_nc(), in_maps, core_ids=list(range(NCORES)), **_RUN_KWARGS
    )
    if _RUN_RESULTS is not None:
        _RUN_RESULTS.append(res)

    out = np.empty((B, S, HID), np.float32)
    for b in range(B):
        out[b] = res.results[2 * b]["out_p"] + res.results[2 * b + 1]["out_p"] + bo
    return out


# test.py can set these to enable tracing / inspect profile results.
_RUN_KWARGS = {}
_RUN_RESULTS = None

